# revision 86
# baseline (speedup 1.0000x reference)
"""Multi-head causal self-attention (torch nn.MultiheadAttention semantics)
on 8 Trainium2 NeuronCores.

Problem: x [2, 2048, 1024], 16 heads, head dim 64, fp32, causal, p_drop=0.

Sharding: 2 batch groups x 4-way head tensor-parallel.
  core c: batch b = c // 4, heads [lane*4, lane*4+4) with lane = c % 4.
The host sums the 4 partial out-projections per batch and adds b_out.

Design (measured ~180-184us vs the 291.9us phase-serial f32r baseline;
run-to-run variance on shared hardware is ~±8-15%):
  - bf16 operands everywhere (PSUM accumulation stays fp32); rel-err gate is
    2e-2, measured 4.2e-3. Halves DMA + SBUF footprint.
  - One software-pipelined span loop (SPAN=256 of sq): flash attention steps
    drive the schedule; q/k/v projections (one span-pair = 512 cols per
    fill, halving the per-column LDWEIGHTS cost) and the out-projection are
    deferred into a GLOBAL filler queue drained one PSUM-fill per attention
    step, so bulk matmul bursts never block the strict-FIFO PE queue ahead
    of the next exp's score matmuls.
  - Scores per head-pair land in a dedicated 2-bank PSUM tensor (separate
    tensors per pair so WAR tracking never couples pair A's next-step
    matmuls to pair B's activation); one strided ACT Exp per pair per step
    (144 calls) keeps the ACT engine saturated while the other pair's
    scores issue on the PE.
  - K=64 score matmuls packed pairwise into array row-groups 0-63/64-127
    via tile_position (second matmul of a pair starts ~4ns after the first).
  - Causal masking is done ON the PE: accumulate-matmuls with an identity
    stationary add a 0/-1e4 mask into the PSUM scores; exp underflows the
    masked slots to exactly 0, so no DVE/gpsimd op sits between exp and PV.
  - PV accumulators: 2 heads share one PSUM bank ([65, 2x256]); banks are
    pre-zeroed with DVE memset and all PV matmuls accumulate (start=False),
    which is scheduler-order-independent (no has_written bank clears). PV
    emission lags scores by 2 steps, and each iteration emits ready work
    (PVs, filler) BEFORE the WAR-waiting score matmuls, so stalls never
    sit ahead of ready work in the strict-FIFO PE queue. The d==1 step's
    fully-masked half (128 cols/head) is skipped in scores, exp, and PV.
  - Softmax denominator via the appended ones-column of v' (row 64 of po);
    DVE copy to a partition-0 tile (custom-DVE ops mishandle base
    partitions), reciprocal_approx_fast, gpsimd partition-broadcast, DVE
    multiply writes normalized OT in bf16.
  - PSUM budget: 4 (scores, 2 tensors) + 2 (PV) + 2 (proj ping-pong) = 8.
  - 28 garbage warmup matmuls at t=0 hold the PE HAM clock-gate open
    through the DMA head; inputs arrive in 9 consolidated DMAs.
"""

import os
from contextlib import ExitStack

import ml_dtypes
import numpy as np

import concourse.bass as bass
import concourse.tile as tile
from concourse import bacc, mybir
from concourse.bass_utils import run_bass_kernel_spmd

F32 = mybir.dt.float32
BF16 = mybir.dt.bfloat16
AF = mybir.ActivationFunctionType

B = 2
S = 2048
DM = 1024
N_HEADS = 16
DH = 64
N_CORES = 8
CPG = 4  # cores per group (tensor-parallel width over heads)
HPC = N_HEADS // CPG  # heads per core (4)
DQ = HPC * DH  # 256
SPAN = 256  # sq span per attention round
NSPAN = S // SPAN  # 8
SB = 128  # sk block
NSB = S // SB  # 16
NDM = DM // 128  # 8 dm row-tiles
NQK = 2 * DQ // 128  # 4 qkT tiles (q01, q23, k01, k23)
VW = DH + 1  # 65: per-head v width incl ones column
XCH = 512  # x load chunk (columns)


DEBUG = bool(os.environ.get("KDBG"))


def _declare_io(nc):
    t = {}
    # consolidated host-side layouts: one DMA per tensor (4 for x), issue
    # overhead on the sync queue is ~600ns per dma_start
    if DEBUG:
        t["dbg_qkT"] = nc.dram_tensor("dbg_qkT", [2 * DQ, S], BF16, kind="ExternalOutput").ap()
        t["dbg_vp"] = nc.dram_tensor("dbg_vp", [128, HPC * VW], BF16, kind="ExternalOutput").ap()
        t["dbg_pt"] = nc.dram_tensor("dbg_pt", [128, HPC * SPAN], BF16, kind="ExternalOutput").ap()
        t["dbg_den"] = nc.dram_tensor("dbg_den", [2, 512], F32, kind="ExternalOutput").ap()
        t["dbg_po"] = nc.dram_tensor("dbg_po", [2 * VW, 512], F32, kind="ExternalOutput").ap()
        t["dbg_pt1"] = nc.dram_tensor("dbg_pt1", [128, HPC * SPAN], BF16, kind="ExternalOutput").ap()
        t["dbg_OT"] = nc.dram_tensor("dbg_OT", [DQ, S], BF16, kind="ExternalOutput").ap()
    # x as [128, NDM, S]: partition p, dm-tile c, seq s  (from xT[c*128+p, s])
    t["xT"] = nc.dram_tensor("xT", [128, NDM, S], BF16, kind="ExternalInput").ap()
    # wqk as [128, NDM, 2*DQ], wv as [128, NDM, DQ] (same per-tile packing)
    t["wqkT"] = nc.dram_tensor("wqkT", [128, NDM * 2 * DQ], BF16, kind="ExternalInput").ap()
    t["wvT"] = nc.dram_tensor("wvT", [128, NDM * DQ], BF16, kind="ExternalInput").ap()
    t["woT"] = nc.dram_tensor("woT", [128, 2 * DM], BF16, kind="ExternalInput").ap()
    t["bqk"] = nc.dram_tensor("bqk", [128, NQK], F32, kind="ExternalInput").ap()
    t["bv"] = nc.dram_tensor("bv", [128, DQ], BF16, kind="ExternalInput").ap()
    t["out"] = nc.dram_tensor("out", [S, DM], BF16, kind="ExternalOutput").ap()
    return t


def _build(ctx: ExitStack, tc: tile.TileContext, io: dict):
    nc = tc.nc

    const = ctx.enter_context(tc.tile_pool(name="const", bufs=1))
    work = ctx.enter_context(tc.tile_pool(name="work", bufs=1))
    psum = ctx.enter_context(tc.tile_pool(name="psum", bufs=1, space="PSUM"))

    # ---- input DMAs, in priority order (one dma_start per tensor/chunk) ----
    wqk_all = const.tile([128, NDM * 2 * DQ], BF16, name="wqk_all")
    nc.sync.dma_start(wqk_all[:], io["wqkT"][:])

    def wqk_s(ob, c):
        o = (c * NQK + ob) * 128
        return wqk_all[:, o : o + 128]

    bqk_all = const.tile([128, NQK], F32, name="bqk_all")
    nc.sync.dma_start(bqk_all[:], io["bqk"][:])
    bqk = [bqk_all[:, c : c + 1] for c in range(NQK)]

    xT_all = const.tile([128, NDM * S], BF16, name="xT_all")
    xT = [xT_all[:, c * S : (c + 1) * S] for c in range(NDM)]
    xT3 = xT_all[:].rearrange("p (c s) -> p c s", s=S)
    io_x3 = io["xT"]
    nc.sync.dma_start(xT3[:, :, 0:XCH], io_x3[:, :, 0:XCH])

    wv_all = const.tile([128, NDM * DQ], BF16, name="wv_all")
    nc.sync.dma_start(wv_all[:], io["wvT"][:])
    wv = [wv_all[:, c * DQ : (c + 1) * DQ] for c in range(NDM)]

    bv = const.tile([128, DQ], BF16, name="bv")
    nc.sync.dma_start(bv[:], io["bv"][:])

    for ch in range(1, S // XCH):
        nc.sync.dma_start(
            xT3[:, :, ch * XCH : (ch + 1) * XCH],
            io_x3[:, :, ch * XCH : (ch + 1) * XCH],
        )
        if ch == 1:
            wo_all = const.tile([128, 2 * DM], BF16, name="wo_all")
            nc.sync.dma_start(wo_all[:], io["woT"][:])
            wo = [wo_all[:, c * DM : (c + 1) * DM] for c in range(DQ // 128)]

    # additive causal masks, applied to the PSUM scores by an accumulating
    # matmul with an identity stationary (out += I.T @ M = M). This keeps
    # masking entirely on the PE: no DVE/gpsimd op between exp and PV.
    # M1 [128, 128]: 0 where col >= row else -1e4 (diagonal block d=0)
    # M2 [128, 256]: cols 0:128 all -1e4; cols 128:256 0 where col-128 >= row
    NEG = -10000.0
    ident = const.tile([128, 128], BF16, name="ident")
    nc.gpsimd.memset(ident[:], 1.0)
    nc.gpsimd.affine_select(
        out=ident[:],
        in_=ident[:],
        compare_op=mybir.AluOpType.is_equal,
        fill=0.0,
        base=0,
        pattern=[[1, 128]],
        channel_multiplier=-1,
    )
    m2 = const.tile([128, 256], BF16, name="m2")
    nc.gpsimd.memset(m2[:], 0.0)
    nc.gpsimd.affine_select(
        out=m2[:],
        in_=m2[:],
        compare_op=mybir.AluOpType.is_ge,
        fill=NEG,
        base=-128,
        pattern=[[1, 256]],
        channel_multiplier=-1,
    )
    m1 = m2[:, 128:256]  # [128,128]: 0 where col >= row else NEG

    # ---- persistent tiles ----
    # qkT tiles: 0=q heads(0,1), 1=q heads(2,3), 2=k heads(0,1), 3=k heads(2,3)
    qkT = [const.tile([128, S], BF16, name=f"qkT{b}") for b in range(NQK)]
    vp = [const.tile([128, HPC * VW], BF16, name=f"vp{sb}") for sb in range(NSB)]
    # OT tile c: rows 0:64 = head 2c, rows 64:128 = head 2c+1 (normalized out^T)
    OT = [const.tile([128, S], BF16, name=f"OT{c}") for c in range(HPC // 2)]

    # persistent PSUM: scores (4 banks; head h in cols [h*512, h*512+256)),
    # po (2 banks; pair p holds head 2p at cols 0:256, head 2p+1 at 256:512)
    # one 2-bank score tensor per head pair, so WAR tracking never couples
    # pair A's next-step matmuls to pair B's activation
    ps2 = [
        psum.tile([128, 1024], F32, name=f"ps2_{p}", tag=f"ps2_{p}", bufs=1)
        for p in range(2)
    ]
    po = [
        psum.tile([VW, 512], F32, name=f"po{p}", tag=f"po{p}", bufs=1) for p in range(2)
    ]

    # HAM warmup: dense dummy matmuls during the DMA head so the PE clock
    # gate opens (4/8 -> 8/8) before the first real projection. Garbage
    # inputs; ps4 is overwritten by the first real scores (start=True).
    warm_src = const.tile([128, 640], BF16, name="warm_src")
    nc.gpsimd.memset(warm_src[:], 0.0)
    zeros65 = const.tile([VW, 512], F32, name="zeros65")
    nc.vector.memset(zeros65[:], 0.0)
    for w in range(28):
        nc.tensor.matmul(
            ps2[w % 2][:, (w % 2) * 512 : (w % 2) * 512 + 512],
            warm_src[:, 0:128],
            warm_src[:, 128:640],
            start=True,
            stop=True,
            skip_group_check=True,
        )

    def outproj_fillers(sp):
        # out projection for span sp's two row blocks, as one-fill closures
        # that get interleaved between attention steps (a bulk emission would
        # sit in the strict-FIFO PE queue ahead of the next span's scores)
        fills = []
        obt = {}

        def fill(qb, nh):
            if nh == 0:
                obt[qb] = work.tile(
                    # 4-deep staging so a straggling output DMA (shared HBM)
                    # never chains back into the proj PSUM slot rotation
                    [128, DM], BF16, name=f"ob_{qb}", tag="ob", bufs=4
                )
            ob_t = obt[qb]
            pot = psum.tile([128, 512], F32, name=f"pot_{qb}_{nh}", tag="proj", bufs=2)
            for c in range(HPC // 2):
                nc.tensor.matmul(
                    pot[:],
                    OT[c][:, qb * 128 : (qb + 1) * 128],
                    wo[c][:, nh * 512 : (nh + 1) * 512],
                    start=(c == 0),
                    stop=(c == HPC // 2 - 1),
                )
            nc.vector.tensor_copy(ob_t[:, nh * 512 : (nh + 1) * 512], pot[:])
            if nh == 1:
                nc.sync.dma_start(io["out"][qb * 128 : (qb + 1) * 128, :], ob_t[:])

        for qb in (2 * sp, 2 * sp + 1):
            for nh in range(2):
                fills.append(lambda qb=qb, nh=nh: fill(qb, nh))
        return fills

    # deferred-work queue, drained one fill per attention step ACROSS spans
    # (a per-span queue overflows on short early spans and bulk-flushes at
    # the boundary, blocking the strict-FIFO PE queue)
    fillers = []

    for sp in range(NSPAN):
        s0 = sp * SPAN
        sq = slice(s0, s0 + SPAN)

        # ---- q/k + v projection for a span-pair (512 cols of q/k at a time:
        # halves the LDWEIGHTS count per projected column). Pair (0,1) is
        # projected in the prologue-like iteration 0; pair (sp+1, sp+2) is
        # emitted on ODD iterations so its PE work spreads over the TWO
        # preceding spans' attention gaps. ----
        def proj_fillers(p0, nspans=2):
            fills = []

            def fill_qk(ob, p0=p0, nspans=nspans):
                sq2 = slice(p0 * SPAN, (p0 + nspans) * SPAN)
                w = nspans * SPAN
                pqk = psum.tile([128, 512], F32, name=f"pqk_{ob}_{p0}", tag="proj", bufs=2)
                for c in range(NDM):
                    nc.tensor.matmul(
                        pqk[:, 0:w],
                        wqk_s(ob, c),
                        xT[c][:, sq2],
                        start=(c == 0),
                        stop=(c == NDM - 1),
                    )
                nc.vector.tensor_scalar_add(qkT[ob][:, sq2], pqk[:, 0:w], bqk[ob][:])

            def fill_v(sb):
                pv = psum.tile([128, DQ], F32, name=f"pv_{sb}", tag="proj", bufs=2)
                for c in range(NDM):
                    nc.tensor.matmul(
                        pv[:],
                        xT[c][:, sb * 128 : (sb + 1) * 128],
                        wv[c][:],
                        start=(c == 0),
                        stop=(c == NDM - 1),
                    )
                vdst = vp[sb][:, 0 : HPC * VW].rearrange("p (h w) -> p h w", w=VW)[
                    :, :, 0:DH
                ]
                nc.vector.tensor_add(
                    vdst,
                    pv[:].rearrange("p (h d) -> p h d", d=DH),
                    bv[:].rearrange("p (h d) -> p h d", d=DH),
                )
                ones_cols = vp[sb][:, DH : HPC * VW : VW]
                nc.vector.memset(ones_cols, 1.0)

            for ob in range(NQK):
                fills.append(lambda ob=ob: fill_qk(ob))
            for sb in range(2 * p0, 2 * p0 + 2 * nspans):
                fills.append(lambda sb=sb: fill_v(sb))
            return fills

        if sp == 0:
            for f in proj_fillers(0):
                f()  # prologue: nothing to interleave with yet
        if sp % 2 == 1 and sp < NSPAN - 1:
            fillers += proj_fillers(sp + 1)
        if sp > 0:
            fillers += outproj_fillers(sp - 1)

        # ---- attention over sk blocks 0..2sp+1 ----
        nsb = 2 * (sp + 1)
        # pre-zero PV accumulator banks via the ACT engine (idle at span
        # boundaries, has a PSUM write port; ACT writes don't set
        # has_written so PV matmuls still accumulate-onto-zero); all PV
        # matmuls accumulate (start=False), order-independent
        for p in range(2):
            nc.scalar.copy(po[p][:], zeros65[:])

        pts = {}

        def emit_scores(sb, sp=sp, s0=s0):
            d = sb - 2 * sp
            off = 128 if d == 1 else 0  # cols 0:128 fully masked on d==1
            for pr in range(2):
                qt = qkT[pr]
                kt = qkT[2 + pr]
                for sub in range(2):  # head 2*pr+sub, array row-group sub
                    r0, r1 = sub * 64, sub * 64 + 64
                    nc.tensor.matmul(
                        ps2[pr][:, sub * 512 + off : sub * 512 + SPAN],
                        kt[r0:r1, sb * 128 : (sb + 1) * 128],
                        qt[r0:r1, s0 + off : s0 + SPAN],
                        start=True,
                        stop=True,
                        tile_position=(sub * 64, 0),
                    )
            if d >= 0:
                # additive triangular mask accumulated into the scores by
                # the PE itself (out += I.T @ m1); masked slots end up at
                # ~-1e4, so exp underflows to exactly 0. The d==1 step's
                # dead half (cols 0:128 per head) is simply never computed,
                # exp'd, or streamed by PV.
                for pr in range(2):
                    for sub in range(2):
                        nc.tensor.matmul(
                            ps2[pr][:, sub * 512 + off : sub * 512 + off + 128],
                            ident[:],
                            m1,
                            start=False,
                            stop=True,
                            skip_group_check=True,
                        )
            pt = work.tile([128, HPC * SPAN], BF16, name=f"pt_{sp}_{sb}", tag="pt", bufs=5)
            pts[sb] = pt
            # two independent exp calls (head pairs) so the ACT engine stays
            # saturated: scores for pair A of step i+1 issue while pair B of
            # step i is still in the activation
            for pr in range(2):
                nc.scalar.activation(
                    pt[:, pr * 2 * SPAN : (pr + 1) * 2 * SPAN].rearrange(
                        "p (h w) -> p h w", w=SPAN
                    )[:, :, off:SPAN],
                    ps2[pr][:].rearrange("p (h w) -> p h w", w=512)[:, :, off:SPAN],
                    AF.Exp,
                    scale=0.125,
                )
            if DEBUG and sp == 0 and sb == 0:
                nc.sync.dma_start(io["dbg_pt"][:], pt[:])
            if DEBUG and sp == 0 and sb == 1:
                nc.sync.dma_start(io["dbg_pt1"][:], pt[:])

        def emit_pvs(sb, nsb=nsb, sp=sp):
            pt = pts.pop(sb)
            off = 128 if sb == 2 * sp + 1 else 0  # d==1: dead half not read
            for h in range(HPC):
                nc.tensor.matmul(
                    po[h // 2][:, (h % 2) * SPAN + off : (h % 2 + 1) * SPAN],
                    vp[sb][:, h * VW : (h + 1) * VW],
                    pt[:, h * SPAN + off : (h + 1) * SPAN],
                    start=False,
                    stop=(sb == nsb - 1),
                    skip_group_check=True,
                )

        # PV lags scores by 2 steps so the first PV of a span (which waits on
        # the po memset behind the previous span's normalize chain) never
        # blocks the strict-FIFO PE queue ahead of ready score matmuls
        # per iteration, emit READY work (PVs, filler) BEFORE the scores:
        # the scores wait on the previous exp (WAR on their score bank), and
        # in the strict-FIFO PE queue everything emitted behind them would
        # inherit that stall
        LAG = 2 if sp < NSPAN - 1 else 1
        for i in range(nsb + LAG):
            if i >= LAG:
                emit_pvs(i - LAG)
            if fillers:
                fillers.pop(0)()
            if i < nsb:
                emit_scores(i)

        # ---- normalize: OT[h-rows, span] = po_v / po_denom ----
        for p in range(2):
            # custom-DVE ops mishandle nonzero base partitions: stage the
            # denominator row (partition 64) to a partition-0 tile first.
            # ACT engine: it is idle at span boundaries while DVE is not.
            den_c = work.tile([1, 512], F32, name=f"den_{p}_{sp}", tag="den", bufs=2)
            nc.scalar.copy(den_c[:], po[p][VW - 1 : VW, :])
            denr = work.tile([1, 512], F32, name=f"denr_{p}_{sp}", tag="denr", bufs=2)
            nc.vector.reciprocal_approx_fast(denr[:], den_c[:])
            if DEBUG and sp == 0:
                nc.sync.dma_start(io["dbg_den"][p : p + 1, :], denr[:])
                po_sb = work.tile([VW, 512], F32, name=f"po_sb_{p}", tag=f"po_sb{p}", bufs=1)
                nc.vector.tensor_copy(po_sb[:], po[p][:])
                nc.sync.dma_start(io["dbg_po"][p * VW : (p + 1) * VW, :], po_sb[:])
            recb = work.tile([DH, 512], F32, name=f"recb_{p}_{sp}", tag="recb", bufs=2)
            nc.gpsimd.partition_broadcast(recb[:], denr[0:1, :])
            for sub in range(2):
                nc.vector.tensor_mul(
                    OT[p][sub * 64 : sub * 64 + 64, sq],
                    po[p][0:DH, sub * SPAN : (sub + 1) * SPAN],
                    recb[:, sub * SPAN : (sub + 1) * SPAN],
                )

    for f in fillers + outproj_fillers(NSPAN - 1):
        f()

    if DEBUG:
        for b in range(NQK):
            nc.sync.dma_start(io["dbg_qkT"][b * 128 : (b + 1) * 128, :], qkT[b][:])
        nc.sync.dma_start(io["dbg_vp"][:], vp[0][:])
        for c in range(HPC // 2):
            nc.sync.dma_start(io["dbg_OT"][c * 128 : (c + 1) * 128, :], OT[c][:])


_NC_CACHE = {}


def _get_compiled():
    if "nc" not in _NC_CACHE:
        nc = bacc.Bacc(
            "TRN2", target_bir_lowering=False, debug=False, num_devices=N_CORES
        )
        io = _declare_io(nc)
        with tile.TileContext(nc) as tc, ExitStack() as ctx:
            _build(ctx, tc, io)
        nc.compile()
        _NC_CACHE["nc"] = nc
    return _NC_CACHE["nc"]


def _bf16(a):
    return np.ascontiguousarray(a.astype(ml_dtypes.bfloat16))


def _prep_core_inputs(x, W_qkv, b_qkv, W_out, b_out, core_id):
    g = core_id // CPG
    lane = core_id % CPG
    h0 = lane * HPC
    r = slice(h0 * DH, (h0 + HPC) * DH)
    Wq = W_qkv[0 * DM : 1 * DM, :][r, :]
    Wk = W_qkv[1 * DM : 2 * DM, :][r, :]
    Wv = W_qkv[2 * DM : 3 * DM, :][r, :]
    bq = b_qkv[0 * DM + h0 * DH : 0 * DM + (h0 + HPC) * DH]
    bk = b_qkv[1 * DM + h0 * DH : 1 * DM + (h0 + HPC) * DH]
    bv_ = b_qkv[2 * DM + h0 * DH : 2 * DM + (h0 + HPC) * DH]
    def tilepack(a):  # [R, C] with R = n*128  ->  [128, n, C]
        n = a.shape[0] // 128
        return a.reshape(n, 128, a.shape[1]).transpose(1, 0, 2)

    return {
        "xT": _bf16(tilepack(x[g].T)),
        "wqkT": _bf16(tilepack(np.concatenate([Wq.T, Wk.T], axis=1)).reshape(128, -1)),
        "wvT": _bf16(tilepack(Wv.T).reshape(128, -1)),
        "woT": _bf16(tilepack(W_out[:, r].T).reshape(128, -1)),
        "bqk": np.ascontiguousarray(
            np.concatenate([bq, bk]).reshape(NQK, 128).T.astype(np.float32)
        ),
        "bv": _bf16(np.broadcast_to(bv_.reshape(1, DQ), (128, DQ))),
    }


def kernel(x, W_qkv, b_qkv, W_out, b_out, _trace=False):
    x = np.asarray(x)
    W_qkv = np.asarray(W_qkv)
    b_qkv = np.asarray(b_qkv)
    W_out = np.asarray(W_out)
    b_out = np.asarray(b_out)

    nc = _get_compiled()
    in_maps = [
        _prep_core_inputs(x, W_qkv, b_qkv, W_out, b_out, c) for c in range(N_CORES)
    ]
    res = run_bass_kernel_spmd(nc, in_maps, list(range(N_CORES)), trace=_trace)

    out = np.empty((B, S, DM), dtype=np.float32)
    for g in range(B):
        acc = res.results[g * CPG]["out"].astype(np.float32)
        for lane in range(1, CPG):
            acc = acc + res.results[g * CPG + lane]["out"].astype(np.float32)
        out[g] = acc + b_out[None, :].astype(np.float32)

    globals()["kernel_last_res"] = res
    if _trace:
        kernel.last_exec_time_ns = res.exec_time_ns
        kernel.last_results = res
    return out
